# revision 10
# baseline (speedup 1.0000x reference)
"""GNN message-passing (NodeModel) kernel for 8 Trainium2 NeuronCores.

Strategy (edge-parallel + node-parallel, zero collectives):
  - Host: sort edges by destination; nodes split into 128-node windows
    (8 cores x 49 windows). Window edges packed into T_SUB=17 subtiles of
    128 slots. Host gathers x[row] per edge and ships feature-major slabs,
    plus per-window selection matrices P (one-hot dest with 1/deg folded
    in) so the device runs pure matmul pipelines.
  - Device per core, per window: edge MLP layer 1 in edge-major layout
    (lhsT = slab subtile), ELU via exp/relu/min identity, then the
    scatter-mean as P^T @ elu1 accumulated in PSUM. Because layer 2 of the
    edge MLP is linear, it commutes with the (linear) mean aggregation:
    w1b is applied to the 128 aggregated node vectors per window instead
    of 2176 edge vectors (17x less work). The elu(x)+1 shift used by the
    ELU identity is corrected through w1b's column sums, folded into the
    node-MLP bias on device.
  - All matmuls run in float32r (full-rate fp32 / TF32-class mode).
  - Node MLP: feature-major, ELU same trick, final layer node-major.

The kernel takes FULL inputs and returns the FULL [50000, 256] output.
"""
import numpy as np

import concourse.bass as bass
import concourse.bacc as bacc
import concourse.tile as tile
import concourse.mybir as mybir
from concourse import bass_utils
from concourse.masks import make_identity

P = 128
N_NODES = 50000
HID = 256
D_OUT = 256
N_CORES = 8

W_WIN = 49            # windows per core
T_SUB = 17            # 128-edge subtiles per window
B = T_SUB * P         # edge slots per window (2176)
EPC = W_WIN * B       # edge slots per core
NPC = W_WIN * P       # node slots per core (6272)
N_PAIR = (T_SUB + 1) // 2   # 9 psum pairs per window (last is half)
N_STT = 4             # chunks using the relu+stt variant (ACT/DVE balance)

F32 = mybir.dt.float32
FR = mybir.dt.float32r

_cache = {}


def _preprocess(x, edge_index, edge_attr):
    x = np.ascontiguousarray(np.asarray(x, dtype=np.float32))
    ea = np.ascontiguousarray(np.asarray(edge_attr, dtype=np.float32))
    ei = np.asarray(edge_index)
    row = ei[0].astype(np.int64)
    col = ei[1].astype(np.int64)
    E = row.shape[0]

    order = np.argsort(col, kind="stable")
    col_s = col[order]
    row_s = row[order]

    win = col_s >> 7
    n_win_tot = N_CORES * W_WIN
    wc = np.bincount(win, minlength=n_win_tot)
    assert wc.max() <= B, f"window overflow: {wc.max()} > {B}"
    wstart = np.zeros(n_win_tot + 1, np.int64)
    np.cumsum(wc, out=wstart[1:])
    rank = np.arange(E, dtype=np.int64) - wstart[win]

    slot = win * B + rank
    TOT = n_win_tot * B

    deg = np.bincount(col, minlength=N_NODES)
    invdeg = np.where(deg > 0, 1.0 / np.maximum(deg, 1), 0.0).astype(np.float32)

    xg = np.zeros((TOT, P), np.float32)
    xg[slot] = x[row_s]
    xslab = xg.reshape(N_CORES, EPC, P).transpose(0, 2, 1).copy()
    del xg

    eag = np.zeros((TOT, P), np.float32)
    eag[slot] = ea[order]
    easlab = eag.reshape(N_CORES, EPC, P).transpose(0, 2, 1).copy()
    del eag

    # colloc[win, p, t]: local dest node of edge slot (t, p), -1 for pads
    colloc = np.full((n_win_tot, T_SUB, P), -1.0, np.float32)
    e_t = (rank >> 7)
    e_p = (rank & 127)
    colloc[win, e_t, e_p] = (col_s - (win << 7)).astype(np.float32)
    colloc = np.ascontiguousarray(
        colloc.transpose(0, 2, 1).reshape(N_CORES, W_WIN, P, T_SUB))
    # invdn[win, n] = 1/deg of local node n
    nodepad = np.zeros(N_CORES * NPC, np.float32)
    nodepad[:N_NODES] = invdeg
    invdn = nodepad.reshape(N_CORES, W_WIN, P)

    xpad = np.zeros((N_CORES * NPC, P), np.float32)
    xpad[:N_NODES] = x
    xT = xpad.reshape(N_CORES, NPC, P).transpose(0, 2, 1).copy()

    return xslab, easlab, colloc, invdn, xT


def _build_program(with_b1a):
    nc = bacc.Bacc("TRN2", target_bir_lowering=False, debug=False,
                   enable_asserts=False)

    xs_d = nc.dram_tensor("xslab", [P, EPC], FR, kind="ExternalInput")
    ea_d = nc.dram_tensor("easlab", [P, EPC], FR, kind="ExternalInput")
    cl_d = nc.dram_tensor("colloc", [W_WIN, P, T_SUB], F32, kind="ExternalInput")
    ivn_d = nc.dram_tensor("invdn", [W_WIN, P], F32, kind="ExternalInput")
    xT_d = nc.dram_tensor("xT", [P, NPC], FR, kind="ExternalInput")
    w1a_d = nc.dram_tensor("w1a", [2 * P, HID], F32, kind="ExternalInput")
    b1a_d = nc.dram_tensor("b1a", [HID], F32, kind="ExternalInput")
    w1b_d = nc.dram_tensor("w1b", [HID, HID], F32, kind="ExternalInput")
    b1b_d = nc.dram_tensor("b1b", [HID], F32, kind="ExternalInput")
    w2a_d = nc.dram_tensor("w2a", [3 * P, HID], F32, kind="ExternalInput")
    b2a_d = nc.dram_tensor("b2a", [HID], F32, kind="ExternalInput")
    w2b_d = nc.dram_tensor("w2b", [HID, D_OUT], F32, kind="ExternalInput")
    b2b_d = nc.dram_tensor("b2b", [D_OUT], F32, kind="ExternalInput")
    out_d = nc.dram_tensor("out", [NPC, D_OUT], F32, kind="ExternalOutput")

    AF = mybir.ActivationFunctionType
    OP = mybir.AluOpType

    with tile.TileContext(nc) as tc:
        with (
            tc.tile_pool(name="const", bufs=1) as cp,
            tc.tile_pool(name="slab", bufs=2) as slabp,
            tc.tile_pool(name="el1", bufs=2) as el1p,
            tc.tile_pool(name="scr", bufs=4) as scrp,
            tc.tile_pool(name="m1", bufs=1) as m1p,
            tc.tile_pool(name="node", bufs=2) as nodep,
            tc.tile_pool(name="outp", bufs=3) as outp,
            tc.tile_pool(name="ps_l1", bufs=4, space="PSUM") as ps_l1,
            tc.tile_pool(name="ps_l2", bufs=2, space="PSUM") as ps_l2,
            tc.tile_pool(name="ps_agg", bufs=2, space="PSUM") as ps_agg,
        ):
            # ================= setup =================
            ident = cp.tile([P, P], F32)
            make_identity(nc, ident[:])

            iota_i = cp.tile([P, B], mybir.dt.int32)
            nc.gpsimd.iota(iota_i[:], pattern=[[0, T_SUB], [1, P]], base=0,
                           channel_multiplier=0)
            iota_rep = cp.tile([P, B], F32)
            nc.vector.tensor_copy(out=iota_rep[:], in_=iota_i[:])

            ones_f = cp.tile([P, 2], F32)
            nc.vector.memset(ones_f[:], 1.0)
            ones_fr = cp.tile([P, 2], FR)
            nc.vector.tensor_copy(out=ones_fr[:], in_=ones_f[:])

            w1a = [cp.tile([P, HID], FR, name=f"w1a{k}") for k in range(2)]
            w1b = [cp.tile([P, HID], FR, name=f"w1b{k}") for k in range(2)]
            w2a = [cp.tile([P, HID], FR, name=f"w2a{k}") for k in range(3)]
            w2b = [cp.tile([P, D_OUT], FR, name=f"w2b{k}") for k in range(2)]
            for k in range(2):
                nc.gpsimd.dma_start(out=w1a[k][:], in_=w1a_d[k * P:(k + 1) * P, :])
                nc.gpsimd.dma_start(out=w1b[k][:], in_=w1b_d[k * P:(k + 1) * P, :])
                nc.gpsimd.dma_start(out=w2b[k][:], in_=w2b_d[k * P:(k + 1) * P, :])
            for k in range(3):
                nc.gpsimd.dma_start(out=w2a[k][:], in_=w2a_d[k * P:(k + 1) * P, :])

            b1b_v = [cp.tile([P, 1], F32, name=f"b1b_v{k}") for k in range(2)]
            b2a_v = [cp.tile([P, 1], F32, name=f"b2a_v{k}") for k in range(2)]
            for k in range(2):
                nc.sync.dma_start(out=b1b_v[k][:], in_=b1b_d[k * P:(k + 1) * P, None])
                nc.sync.dma_start(out=b2a_v[k][:], in_=b2a_d[k * P:(k + 1) * P, None])
            b2b_row = cp.tile([1, D_OUT], F32)
            nc.sync.dma_start(out=b2b_row[:], in_=b2b_d[None, :])

            if with_b1a:
                # b1a as a [1, 2*HID] row (doubled) for the K=1 bias matmul
                b1a_row2 = cp.tile([1, 2 * HID], FR)
                for rep in range(2):
                    nc.gpsimd.dma_start(
                        out=b1a_row2[:, rep * HID:(rep + 1) * HID],
                        in_=b1a_d[None, :])
                ones_row_f = cp.tile([1, P], F32)
                nc.vector.memset(ones_row_f[:], 1.0)
                ones_row = cp.tile([1, P], FR)
                nc.vector.tensor_copy(out=ones_row[:], in_=ones_row_f[:])

            # cvec = b1b - colsum(w1b)  (for the elu+1 shift correction)
            cvec = [cp.tile([P, 2], FR, name=f"cvec{j}") for j in range(2)]
            for j in range(2):
                cs_ps = ps_l2.tile([P, 2], F32, tag="ps_l2")
                for k in range(2):
                    nc.tensor.matmul(out=cs_ps[:], lhsT=w1b[k][:, j * P:(j + 1) * P],
                                     rhs=ones_fr[:], start=(k == 0), stop=(k == 1))
                nc.vector.tensor_tensor(out=cvec[j][:],
                                        in0=b1b_v[j][:].to_broadcast([P, 2]),
                                        in1=cs_ps[:], op=OP.subtract)

            # b2a_eff = b2a + w2a[128:384].T @ cvec
            b2a_eff = [cp.tile([P, 1], F32, name=f"b2a_eff{m}") for m in range(2)]
            b2a_eff_p1 = [cp.tile([P, 1], F32, name=f"b2a_eff_p1{m}") for m in range(2)]
            for m in range(2):
                be_ps = ps_l2.tile([P, 2], F32, tag="ps_l2")
                for k in range(2):
                    nc.tensor.matmul(out=be_ps[:], lhsT=w2a[1 + k][:, m * P:(m + 1) * P],
                                     rhs=cvec[k][:], start=(k == 0), stop=(k == 1))
                nc.vector.tensor_tensor(out=b2a_eff[m][:], in0=b2a_v[m][:],
                                        in1=be_ps[:, 0:1], op=OP.add)
                nc.vector.tensor_scalar_add(out=b2a_eff_p1[m][:], in0=b2a_eff[m][:],
                                            scalar1=1.0)

            # b2b_eff broadcast [P, 256] = b2b - colsum(w2b)
            csb_ps = ps_l2.tile([1, D_OUT], F32, tag="ps_l2")
            for k in range(2):
                nc.tensor.matmul(out=csb_ps[:], lhsT=ones_fr[:, 0:1], rhs=w2b[k][:],
                                 start=(k == 0), stop=(k == 1))
            b2b_eff_row = cp.tile([1, D_OUT], FR)
            nc.vector.tensor_tensor(out=b2b_eff_row[:], in0=b2b_row[:], in1=csb_ps[:],
                                    op=OP.subtract)
            onesc_f = cp.tile([1, P], F32, name="onesc_f")
            nc.vector.memset(onesc_f[:], 1.0)
            onesc = cp.tile([1, P], FR, name="onesc")
            nc.vector.tensor_copy(out=onesc[:], in_=onesc_f[:])
            bb_ps = ps_l1.tile([P, D_OUT], F32, tag="ps_l1")
            nc.tensor.matmul(out=bb_ps[:], lhsT=onesc[:], rhs=b2b_eff_row[:],
                             start=True, stop=True)
            b2b_bc = cp.tile([P, D_OUT], F32)
            nc.scalar.copy(out=b2b_bc[:], in_=bb_ps[:])

            m1_buf = m1p.tile([P, W_WIN * HID], F32)

            # ================= per-window edge phase =================
            def edge_window(w):
                xs = slabp.tile([P, B], FR, tag="xs", name=f"xs{w}")
                nc.sync.dma_start(out=xs[:], in_=xs_d[:, w * B:(w + 1) * B])
                eas = slabp.tile([P, B], FR, tag="eas", name=f"eas{w}")
                nc.sync.dma_start(out=eas[:], in_=ea_d[:, w * B:(w + 1) * B])
                cl = slabp.tile([P, T_SUB], F32, tag="cl", name=f"cl{w}")
                nc.sync.dma_start(out=cl[:], in_=cl_d[w, :, :])
                ivn = slabp.tile([P, 1], F32, tag="ivn", name=f"ivn{w}")
                nc.sync.dma_start(out=ivn[:], in_=ivn_d[w, :, None])
                psl = slabp.tile([P, B], FR, tag="psl", name=f"psl{w}")
                nc.vector.tensor_tensor(
                    out=psl[:].rearrange("p (t n) -> p t n", n=P),
                    in0=iota_rep[:].rearrange("p (t n) -> p t n", n=P),
                    in1=cl[:, :, None].to_broadcast([P, T_SUB, P]),
                    op=OP.is_equal)

                el1 = el1p.tile([P, T_SUB * HID], FR, tag="el1", name=f"el1_{w}")
                for pr in range(N_PAIR):
                    t0 = 2 * pr
                    nsub = 1 if t0 == T_SUB - 1 else 2
                    pw = nsub * HID
                    h1 = ps_l1.tile([P, 512], F32, tag="ps_l1", name=f"h1_{w}_{pr}")
                    if with_b1a:
                        nc.tensor.matmul(out=h1[:, :pw], lhsT=ones_row[:],
                                         rhs=b1a_row2[:, :pw], start=True, stop=False)
                    for ti in range(nsub):
                        t = t0 + ti
                        for k in range(2):
                            slab = xs if k == 0 else eas
                            nc.tensor.matmul(
                                out=h1[:, ti * HID:(ti + 1) * HID],
                                lhsT=slab[:, t * P:(t + 1) * P],
                                rhs=(w1a[k][:]),
                                start=(k == 0 and not with_b1a),
                                stop=(k == 1))
                    seg = el1[:, t0 * HID: t0 * HID + pw]
                    if pr < N_PAIR - N_STT:
                        # variant A: exp (ACT) -> r1 (DVE ts) -> min (DVE tt)
                        nc.scalar.activation(out=seg, in_=h1[:, :pw], func=AF.Exp)
                        r1 = scrp.tile([P, 512], F32, tag="scr", name=f"r1_{w}_{pr}")
                        nc.vector.tensor_scalar(
                            out=r1[:, :pw], in0=h1[:, :pw],
                            scalar1=1.0, scalar2=1.0, op0=OP.add, op1=OP.max)
                        nc.vector.tensor_tensor(out=seg, in0=seg, in1=r1[:, :pw],
                                                op=OP.min)
                    else:
                        # variant B: exp (ACT) + relu (ACT) -> stt (DVE)
                        ex = scrp.tile([P, 512], F32, tag="scr", name=f"ex_{w}_{pr}")
                        nc.scalar.activation(out=ex[:, :pw], in_=h1[:, :pw], func=AF.Exp)
                        rl = scrp.tile([P, 512], F32, tag="scr", name=f"rl_{w}_{pr}")
                        nc.scalar.activation(out=rl[:, :pw], in_=h1[:, :pw], func=AF.Relu)
                        nc.vector.scalar_tensor_tensor(
                            out=seg, in0=rl[:, :pw], scalar=1.0, in1=ex[:, :pw],
                            op0=OP.add, op1=OP.min)

                m1 = ps_agg.tile([P, HID], F32, tag="ps_agg", name=f"m1_{w}")
                for t in range(T_SUB):
                    nc.tensor.matmul(out=m1[:],
                                     lhsT=psl[:, t * P:(t + 1) * P],
                                     rhs=el1[:, t * HID:(t + 1) * HID],
                                     start=(t == 0), stop=(t == T_SUB - 1))
                nc.scalar.activation(out=m1_buf[:, w * HID:(w + 1) * HID], in_=m1[:],
                                     func=AF.Identity, scale=ivn[:, :1])

            # ================= per-group node phase =================
            def node_group(ws):
                ncols = P * len(ws)
                base = ws[0] * P
                gi = ws[0] // 4
                xz = nodep.tile([P, 512], FR, tag="xz", name=f"xz{gi}")
                nc.sync.dma_start(out=xz[:, :ncols], in_=xT_d[:, base:base + ncols])
                # transpose m1 -> feature-major m1f
                m1f = [nodep.tile([P, 512], FR, tag=f"m1f{fh}", name=f"m1f{fh}_{gi}")
                       for fh in range(2)]
                for wi, w in enumerate(ws):
                    for fh in range(2):
                        trp = ps_l2.tile([P, P], F32, tag="ps_l2",
                                         name=f"trp{gi}_{wi}_{fh}")
                        nc.tensor.transpose(
                            out=trp[:],
                            in_=m1_buf[:, w * HID + fh * P: w * HID + (fh + 1) * P],
                            identity=ident[:])
                        nc.scalar.copy(out=m1f[fh][:, wi * P:(wi + 1) * P], in_=trp[:])
                # mean = w1b^T @ m1f  (edge-MLP layer 2 on aggregated nodes)
                zf = [nodep.tile([P, 512], FR, tag=f"zf{m}", name=f"zf{m}_{gi}")
                      for m in range(2)]
                for m in range(2):
                    mps = ps_l1.tile([P, 512], F32, tag="ps_l1", name=f"mps{gi}_{m}")
                    for k in range(2):
                        nc.tensor.matmul(out=mps[:, :ncols],
                                         lhsT=w1b[k][:, m * P:(m + 1) * P],
                                         rhs=m1f[k][:, :ncols],
                                         start=(k == 0), stop=(k == 1))
                    nc.scalar.copy(out=zf[m][:, :ncols], in_=mps[:, :ncols])
                # node MLP layer 1
                el3 = [nodep.tile([P, 512], FR, tag=f"el3{m}", name=f"el3{m}_{gi}")
                       for m in range(2)]
                for m in range(2):
                    h3 = ps_l1.tile([P, 512], F32, tag="ps_l1", name=f"h3_{gi}_{m}")
                    for k in range(3):
                        rhs = (xz if k == 0 else zf[k - 1])[:, :ncols]
                        nc.tensor.matmul(out=h3[:, :ncols],
                                         lhsT=w2a[k][:, m * P:(m + 1) * P],
                                         rhs=rhs, start=(k == 0), stop=(k == 2))
                    nc.scalar.activation(out=el3[m][:, :ncols], in_=h3[:, :ncols],
                                         func=AF.Exp, bias=b2a_eff[m][:, :1], scale=1.0)
                    r3 = scrp.tile([P, 512], F32, tag="scr", name=f"r3_{gi}_{m}")
                    nc.vector.tensor_scalar(
                        out=r3[:, :ncols], in0=h3[:, :ncols],
                        scalar1=b2a_eff_p1[m][:, :1], scalar2=1.0,
                        op0=OP.add, op1=OP.max)
                    nc.vector.tensor_tensor(out=el3[m][:, :ncols],
                                            in0=el3[m][:, :ncols], in1=r3[:, :ncols],
                                            op=OP.min)
                # node MLP layer 2 + bias, store
                for si, w in enumerate(ws):
                    o_ps = ps_l2.tile([P, D_OUT], F32, tag="ps_l2",
                                      name=f"ops{gi}_{si}")
                    for k in range(2):
                        nc.tensor.matmul(out=o_ps[:],
                                         lhsT=el3[k][:, si * P:(si + 1) * P],
                                         rhs=w2b[k][:], start=(k == 0), stop=(k == 1))
                    ob = outp.tile([P, D_OUT], F32, tag="ob", name=f"ob{gi}_{si}")
                    nc.vector.tensor_tensor(out=ob[:], in0=o_ps[:], in1=b2b_bc[:],
                                            op=OP.add)
                    nc.sync.dma_start(out=out_d[w * P:(w + 1) * P, :], in_=ob[:])

            for w in range(W_WIN):
                edge_window(w)
                if w % 4 == 3:
                    node_group(list(range(w - 3, w + 1)))
            node_group([W_WIN - 1])

    nc.compile()
    return nc


def kernel(x, edge_index, edge_attr, u, batch,
           w1a, b1a, w1b, b1b, w2a, b2a, w2b, b2b, _trace=False):
    xslab, easlab, colloc, invdn, xT = _preprocess(x, edge_index, edge_attr)

    b1a = np.ascontiguousarray(np.asarray(b1a, np.float32))
    with_b1a = bool(np.any(b1a != 0))
    key = ("nc", with_b1a)
    if key not in _cache:
        _cache[key] = _build_program(with_b1a)
    nc = _cache[key]

    wmap = dict(
        w1a=np.ascontiguousarray(np.asarray(w1a, np.float32)),
        b1a=b1a,
        w1b=np.ascontiguousarray(np.asarray(w1b, np.float32)),
        b1b=np.ascontiguousarray(np.asarray(b1b, np.float32)),
        w2a=np.ascontiguousarray(np.asarray(w2a, np.float32)),
        b2a=np.ascontiguousarray(np.asarray(b2a, np.float32)),
        w2b=np.ascontiguousarray(np.asarray(w2b, np.float32)),
        b2b=np.ascontiguousarray(np.asarray(b2b, np.float32)),
    )
    in_maps = []
    for c in range(N_CORES):
        m = dict(wmap)
        m["xslab"] = xslab[c]
        m["easlab"] = easlab[c]
        m["colloc"] = colloc[c]
        m["invdn"] = invdn[c]
        m["xT"] = xT[c]
        in_maps.append(m)

    res = bass_utils.run_bass_kernel_spmd(
        nc, in_maps, core_ids=list(range(N_CORES)), trace=_trace)
    kernel._last_res = res

    out = np.empty((N_CORES * NPC, D_OUT), np.float32)
    for c in range(N_CORES):
        out[c * NPC:(c + 1) * NPC] = res.results[c]["out"]
    return out[:N_NODES]


# revision 11
# speedup vs baseline: 1.0466x; 1.0466x over previous
"""GNN message-passing (NodeModel) kernel for 8 Trainium2 NeuronCores.

Strategy (edge-parallel + node-parallel, zero collectives):
  - Host: sort edges by destination; nodes split into 128-node windows
    (8 cores x 49 windows). Window edges packed into T_SUB=17 subtiles of
    128 slots. Host gathers x[row] per edge and ships feature-major slabs,
    plus per-window selection matrices P (one-hot dest with 1/deg folded
    in) so the device runs pure matmul pipelines.
  - Device per core, per window: edge MLP layer 1 in edge-major layout
    (lhsT = slab subtile), ELU via exp/relu/min identity, then the
    scatter-mean as P^T @ elu1 accumulated in PSUM. Because layer 2 of the
    edge MLP is linear, it commutes with the (linear) mean aggregation:
    w1b is applied to the 128 aggregated node vectors per window instead
    of 2176 edge vectors (17x less work). The elu(x)+1 shift used by the
    ELU identity is corrected through w1b's column sums, folded into the
    node-MLP bias on device.
  - All matmuls run in float32r (full-rate fp32 / TF32-class mode).
  - Node MLP: feature-major, ELU same trick, final layer node-major.

The kernel takes FULL inputs and returns the FULL [50000, 256] output.
"""
import numpy as np

import concourse.bass as bass
import concourse.bacc as bacc
import concourse.tile as tile
import concourse.mybir as mybir
from concourse import bass_utils
from concourse.masks import make_identity

P = 128
N_NODES = 50000
HID = 256
D_OUT = 256
N_CORES = 8

W_WIN = 49            # windows per core
T_SUB = 17            # 128-edge subtiles per window
B = T_SUB * P         # edge slots per window (2176)
EPC = W_WIN * B       # edge slots per core
NPC = W_WIN * P       # node slots per core (6272)
N_PAIR = (T_SUB + 1) // 2   # 9 psum pairs per window (last is half)
N_STT = 4             # chunks using the relu+stt variant (ACT/DVE balance)

F32 = mybir.dt.float32
FR = mybir.dt.float32r

_cache = {}


def _preprocess(x, edge_index, edge_attr):
    x = np.ascontiguousarray(np.asarray(x, dtype=np.float32))
    ea = np.ascontiguousarray(np.asarray(edge_attr, dtype=np.float32))
    ei = np.asarray(edge_index)
    row = ei[0].astype(np.int64)
    col = ei[1].astype(np.int64)
    E = row.shape[0]

    order = np.argsort(col, kind="stable")
    col_s = col[order]
    row_s = row[order]

    win = col_s >> 7
    n_win_tot = N_CORES * W_WIN
    wc = np.bincount(win, minlength=n_win_tot)
    assert wc.max() <= B, f"window overflow: {wc.max()} > {B}"
    wstart = np.zeros(n_win_tot + 1, np.int64)
    np.cumsum(wc, out=wstart[1:])
    rank = np.arange(E, dtype=np.int64) - wstart[win]

    slot = win * B + rank
    TOT = n_win_tot * B

    deg = np.bincount(col, minlength=N_NODES)
    invdeg = np.where(deg > 0, 1.0 / np.maximum(deg, 1), 0.0).astype(np.float32)

    xg = np.zeros((TOT, P), np.float32)
    xg[slot] = x[row_s]
    xslab = xg.reshape(N_CORES, EPC, P).transpose(0, 2, 1).copy()
    del xg

    eag = np.zeros((TOT, P), np.float32)
    eag[slot] = ea[order]
    easlab = eag.reshape(N_CORES, EPC, P).transpose(0, 2, 1).copy()
    del eag

    # P_t [128 edge, 128 node] one-hot (uint8), 1/deg applied at m1 copy
    psl2 = np.zeros((n_win_tot, T_SUB, P, P), np.uint8)
    e_t = (rank >> 7)
    e_p = (rank & 127)
    psl2[win, e_t, e_p, col_s - (win << 7)] = 1
    pslab8 = np.ascontiguousarray(
        psl2.transpose(0, 2, 1, 3).reshape(N_CORES, W_WIN, P, T_SUB * P))
    del psl2
    # invdn[win, n] = 1/deg of local node n
    nodepad = np.zeros(N_CORES * NPC, np.float32)
    nodepad[:N_NODES] = invdeg
    invdn = nodepad.reshape(N_CORES, W_WIN, P)

    xpad = np.zeros((N_CORES * NPC, P), np.float32)
    xpad[:N_NODES] = x
    xT = xpad.reshape(N_CORES, NPC, P).transpose(0, 2, 1).copy()

    return xslab, easlab, pslab8, invdn, xT


def _build_program(with_b1a):
    nc = bacc.Bacc("TRN2", target_bir_lowering=False, debug=False,
                   enable_asserts=False)

    xs_d = nc.dram_tensor("xslab", [P, EPC], FR, kind="ExternalInput")
    ea_d = nc.dram_tensor("easlab", [P, EPC], FR, kind="ExternalInput")
    ps_d = nc.dram_tensor("pslab8", [W_WIN, P, B], mybir.dt.uint8, kind="ExternalInput")
    ivn_d = nc.dram_tensor("invdn", [W_WIN, P], F32, kind="ExternalInput")
    xT_d = nc.dram_tensor("xT", [P, NPC], FR, kind="ExternalInput")
    w1a_d = nc.dram_tensor("w1a", [2 * P, HID], F32, kind="ExternalInput")
    b1a_d = nc.dram_tensor("b1a", [HID], F32, kind="ExternalInput")
    w1b_d = nc.dram_tensor("w1b", [HID, HID], F32, kind="ExternalInput")
    b1b_d = nc.dram_tensor("b1b", [HID], F32, kind="ExternalInput")
    w2a_d = nc.dram_tensor("w2a", [3 * P, HID], F32, kind="ExternalInput")
    b2a_d = nc.dram_tensor("b2a", [HID], F32, kind="ExternalInput")
    w2b_d = nc.dram_tensor("w2b", [HID, D_OUT], F32, kind="ExternalInput")
    b2b_d = nc.dram_tensor("b2b", [D_OUT], F32, kind="ExternalInput")
    out_d = nc.dram_tensor("out", [NPC, D_OUT], F32, kind="ExternalOutput")

    AF = mybir.ActivationFunctionType
    OP = mybir.AluOpType

    with tile.TileContext(nc) as tc:
        with (
            tc.tile_pool(name="const", bufs=1) as cp,
            tc.tile_pool(name="slab", bufs=2) as slabp,
            tc.tile_pool(name="el1", bufs=2) as el1p,
            tc.tile_pool(name="scr", bufs=4) as scrp,
            tc.tile_pool(name="m1", bufs=1) as m1p,
            tc.tile_pool(name="node", bufs=2) as nodep,
            tc.tile_pool(name="outp", bufs=3) as outp,
            tc.tile_pool(name="ps_l1", bufs=4, space="PSUM") as ps_l1,
            tc.tile_pool(name="ps_l2", bufs=2, space="PSUM") as ps_l2,
            tc.tile_pool(name="ps_agg", bufs=2, space="PSUM") as ps_agg,
        ):
            # ================= setup =================
            ident = cp.tile([P, P], F32)
            make_identity(nc, ident[:])

            ones_f = cp.tile([P, 2], F32)
            nc.vector.memset(ones_f[:], 1.0)
            ones_fr = cp.tile([P, 2], FR)
            nc.vector.tensor_copy(out=ones_fr[:], in_=ones_f[:])

            w1a = [cp.tile([P, HID], FR, name=f"w1a{k}") for k in range(2)]
            w1b = [cp.tile([P, HID], FR, name=f"w1b{k}") for k in range(2)]
            w2a = [cp.tile([P, HID], FR, name=f"w2a{k}") for k in range(3)]
            w2b = [cp.tile([P, D_OUT], FR, name=f"w2b{k}") for k in range(2)]
            for k in range(2):
                nc.gpsimd.dma_start(out=w1a[k][:], in_=w1a_d[k * P:(k + 1) * P, :])
                nc.gpsimd.dma_start(out=w1b[k][:], in_=w1b_d[k * P:(k + 1) * P, :])
                nc.gpsimd.dma_start(out=w2b[k][:], in_=w2b_d[k * P:(k + 1) * P, :])
            for k in range(3):
                nc.gpsimd.dma_start(out=w2a[k][:], in_=w2a_d[k * P:(k + 1) * P, :])

            b1b_v = [cp.tile([P, 1], F32, name=f"b1b_v{k}") for k in range(2)]
            b2a_v = [cp.tile([P, 1], F32, name=f"b2a_v{k}") for k in range(2)]
            for k in range(2):
                nc.sync.dma_start(out=b1b_v[k][:], in_=b1b_d[k * P:(k + 1) * P, None])
                nc.sync.dma_start(out=b2a_v[k][:], in_=b2a_d[k * P:(k + 1) * P, None])
            b2b_row = cp.tile([1, D_OUT], F32)
            nc.sync.dma_start(out=b2b_row[:], in_=b2b_d[None, :])

            if with_b1a:
                # b1a as a [1, 2*HID] row (doubled) for the K=1 bias matmul
                b1a_row2 = cp.tile([1, 2 * HID], FR)
                for rep in range(2):
                    nc.gpsimd.dma_start(
                        out=b1a_row2[:, rep * HID:(rep + 1) * HID],
                        in_=b1a_d[None, :])
                ones_row_f = cp.tile([1, P], F32)
                nc.vector.memset(ones_row_f[:], 1.0)
                ones_row = cp.tile([1, P], FR)
                nc.vector.tensor_copy(out=ones_row[:], in_=ones_row_f[:])

            # cvec = b1b - colsum(w1b)  (for the elu+1 shift correction)
            cvec = [cp.tile([P, 2], FR, name=f"cvec{j}") for j in range(2)]
            for j in range(2):
                cs_ps = ps_l2.tile([P, 2], F32, tag="ps_l2")
                for k in range(2):
                    nc.tensor.matmul(out=cs_ps[:], lhsT=w1b[k][:, j * P:(j + 1) * P],
                                     rhs=ones_fr[:], start=(k == 0), stop=(k == 1))
                nc.vector.tensor_tensor(out=cvec[j][:],
                                        in0=b1b_v[j][:].to_broadcast([P, 2]),
                                        in1=cs_ps[:], op=OP.subtract)

            # b2a_eff = b2a + w2a[128:384].T @ cvec
            b2a_eff = [cp.tile([P, 1], F32, name=f"b2a_eff{m}") for m in range(2)]
            b2a_eff_p1 = [cp.tile([P, 1], F32, name=f"b2a_eff_p1{m}") for m in range(2)]
            for m in range(2):
                be_ps = ps_l2.tile([P, 2], F32, tag="ps_l2")
                for k in range(2):
                    nc.tensor.matmul(out=be_ps[:], lhsT=w2a[1 + k][:, m * P:(m + 1) * P],
                                     rhs=cvec[k][:], start=(k == 0), stop=(k == 1))
                nc.vector.tensor_tensor(out=b2a_eff[m][:], in0=b2a_v[m][:],
                                        in1=be_ps[:, 0:1], op=OP.add)
                nc.vector.tensor_scalar_add(out=b2a_eff_p1[m][:], in0=b2a_eff[m][:],
                                            scalar1=1.0)

            # b2b_eff broadcast [P, 256] = b2b - colsum(w2b)
            csb_ps = ps_l2.tile([1, D_OUT], F32, tag="ps_l2")
            for k in range(2):
                nc.tensor.matmul(out=csb_ps[:], lhsT=ones_fr[:, 0:1], rhs=w2b[k][:],
                                 start=(k == 0), stop=(k == 1))
            b2b_eff_row = cp.tile([1, D_OUT], FR)
            nc.vector.tensor_tensor(out=b2b_eff_row[:], in0=b2b_row[:], in1=csb_ps[:],
                                    op=OP.subtract)
            onesc_f = cp.tile([1, P], F32, name="onesc_f")
            nc.vector.memset(onesc_f[:], 1.0)
            onesc = cp.tile([1, P], FR, name="onesc")
            nc.vector.tensor_copy(out=onesc[:], in_=onesc_f[:])
            bb_ps = ps_l1.tile([P, D_OUT], F32, tag="ps_l1")
            nc.tensor.matmul(out=bb_ps[:], lhsT=onesc[:], rhs=b2b_eff_row[:],
                             start=True, stop=True)
            b2b_bc = cp.tile([P, D_OUT], F32)
            nc.scalar.copy(out=b2b_bc[:], in_=bb_ps[:])

            m1_buf = m1p.tile([P, W_WIN * HID], F32)

            # ================= per-window edge phase =================
            def edge_window(w):
                xs = slabp.tile([P, B], FR, tag="xs", name=f"xs{w}")
                nc.sync.dma_start(out=xs[:], in_=xs_d[:, w * B:(w + 1) * B])
                eas = slabp.tile([P, B], FR, tag="eas", name=f"eas{w}")
                nc.sync.dma_start(out=eas[:], in_=ea_d[:, w * B:(w + 1) * B])
                ivn = slabp.tile([P, 1], F32, tag="ivn", name=f"ivn{w}")
                nc.sync.dma_start(out=ivn[:], in_=ivn_d[w, :, None])
                psl = slabp.tile([P, B], FR, tag="psl", name=f"psl{w}")
                nc.gpsimd.dma_start(out=psl[:], in_=ps_d[w, :, :])

                el1 = el1p.tile([P, T_SUB * HID], FR, tag="el1", name=f"el1_{w}")
                for pr in range(N_PAIR):
                    t0 = 2 * pr
                    nsub = 1 if t0 == T_SUB - 1 else 2
                    pw = nsub * HID
                    h1 = ps_l1.tile([P, 512], F32, tag="ps_l1", name=f"h1_{w}_{pr}")
                    if with_b1a:
                        nc.tensor.matmul(out=h1[:, :pw], lhsT=ones_row[:],
                                         rhs=b1a_row2[:, :pw], start=True, stop=False)
                    for ti in range(nsub):
                        t = t0 + ti
                        for k in range(2):
                            slab = xs if k == 0 else eas
                            nc.tensor.matmul(
                                out=h1[:, ti * HID:(ti + 1) * HID],
                                lhsT=slab[:, t * P:(t + 1) * P],
                                rhs=(w1a[k][:]),
                                start=(k == 0 and not with_b1a),
                                stop=(k == 1))
                    seg = el1[:, t0 * HID: t0 * HID + pw]
                    if pr < N_PAIR - N_STT:
                        # variant A: exp (ACT) -> r1 (DVE ts) -> min (DVE tt)
                        nc.scalar.activation(out=seg, in_=h1[:, :pw], func=AF.Exp)
                        r1 = scrp.tile([P, 512], F32, tag="scr", name=f"r1_{w}_{pr}")
                        nc.vector.tensor_scalar(
                            out=r1[:, :pw], in0=h1[:, :pw],
                            scalar1=1.0, scalar2=1.0, op0=OP.add, op1=OP.max)
                        nc.vector.tensor_tensor(out=seg, in0=seg, in1=r1[:, :pw],
                                                op=OP.min)
                    else:
                        # variant B: exp (ACT) + relu (ACT) -> stt (DVE)
                        ex = scrp.tile([P, 512], F32, tag="scr", name=f"ex_{w}_{pr}")
                        nc.scalar.activation(out=ex[:, :pw], in_=h1[:, :pw], func=AF.Exp)
                        rl = scrp.tile([P, 512], F32, tag="scr", name=f"rl_{w}_{pr}")
                        nc.scalar.activation(out=rl[:, :pw], in_=h1[:, :pw], func=AF.Relu)
                        nc.vector.scalar_tensor_tensor(
                            out=seg, in0=rl[:, :pw], scalar=1.0, in1=ex[:, :pw],
                            op0=OP.add, op1=OP.min)

                m1 = ps_agg.tile([P, HID], F32, tag="ps_agg", name=f"m1_{w}")
                for t in range(T_SUB):
                    nc.tensor.matmul(out=m1[:],
                                     lhsT=psl[:, t * P:(t + 1) * P],
                                     rhs=el1[:, t * HID:(t + 1) * HID],
                                     start=(t == 0), stop=(t == T_SUB - 1))
                nc.scalar.activation(out=m1_buf[:, w * HID:(w + 1) * HID], in_=m1[:],
                                     func=AF.Identity, scale=ivn[:, :1])

            # ================= per-group node phase =================
            def node_group(ws):
                ncols = P * len(ws)
                base = ws[0] * P
                gi = ws[0] // 4
                xz = nodep.tile([P, 512], FR, tag="xz", name=f"xz{gi}")
                nc.sync.dma_start(out=xz[:, :ncols], in_=xT_d[:, base:base + ncols])
                # transpose m1 -> feature-major m1f
                m1f = [nodep.tile([P, 512], FR, tag=f"m1f{fh}", name=f"m1f{fh}_{gi}")
                       for fh in range(2)]
                for wi, w in enumerate(ws):
                    for fh in range(2):
                        trp = ps_l2.tile([P, P], F32, tag="ps_l2",
                                         name=f"trp{gi}_{wi}_{fh}")
                        nc.tensor.transpose(
                            out=trp[:],
                            in_=m1_buf[:, w * HID + fh * P: w * HID + (fh + 1) * P],
                            identity=ident[:])
                        nc.scalar.copy(out=m1f[fh][:, wi * P:(wi + 1) * P], in_=trp[:])
                # mean = w1b^T @ m1f  (edge-MLP layer 2 on aggregated nodes)
                zf = [nodep.tile([P, 512], FR, tag=f"zf{m}", name=f"zf{m}_{gi}")
                      for m in range(2)]
                for m in range(2):
                    mps = ps_l1.tile([P, 512], F32, tag="ps_l1", name=f"mps{gi}_{m}")
                    for k in range(2):
                        nc.tensor.matmul(out=mps[:, :ncols],
                                         lhsT=w1b[k][:, m * P:(m + 1) * P],
                                         rhs=m1f[k][:, :ncols],
                                         start=(k == 0), stop=(k == 1))
                    nc.scalar.copy(out=zf[m][:, :ncols], in_=mps[:, :ncols])
                # node MLP layer 1
                el3 = [nodep.tile([P, 512], FR, tag=f"el3{m}", name=f"el3{m}_{gi}")
                       for m in range(2)]
                for m in range(2):
                    h3 = ps_l1.tile([P, 512], F32, tag="ps_l1", name=f"h3_{gi}_{m}")
                    for k in range(3):
                        rhs = (xz if k == 0 else zf[k - 1])[:, :ncols]
                        nc.tensor.matmul(out=h3[:, :ncols],
                                         lhsT=w2a[k][:, m * P:(m + 1) * P],
                                         rhs=rhs, start=(k == 0), stop=(k == 2))
                    nc.scalar.activation(out=el3[m][:, :ncols], in_=h3[:, :ncols],
                                         func=AF.Exp, bias=b2a_eff[m][:, :1], scale=1.0)
                    r3 = scrp.tile([P, 512], F32, tag="scr", name=f"r3_{gi}_{m}")
                    nc.vector.tensor_scalar(
                        out=r3[:, :ncols], in0=h3[:, :ncols],
                        scalar1=b2a_eff_p1[m][:, :1], scalar2=1.0,
                        op0=OP.add, op1=OP.max)
                    nc.vector.tensor_tensor(out=el3[m][:, :ncols],
                                            in0=el3[m][:, :ncols], in1=r3[:, :ncols],
                                            op=OP.min)
                # node MLP layer 2 + bias, store
                for si, w in enumerate(ws):
                    o_ps = ps_l2.tile([P, D_OUT], F32, tag="ps_l2",
                                      name=f"ops{gi}_{si}")
                    for k in range(2):
                        nc.tensor.matmul(out=o_ps[:],
                                         lhsT=el3[k][:, si * P:(si + 1) * P],
                                         rhs=w2b[k][:], start=(k == 0), stop=(k == 1))
                    ob = outp.tile([P, D_OUT], F32, tag="ob", name=f"ob{gi}_{si}")
                    nc.vector.tensor_tensor(out=ob[:], in0=o_ps[:], in1=b2b_bc[:],
                                            op=OP.add)
                    nc.sync.dma_start(out=out_d[w * P:(w + 1) * P, :], in_=ob[:])

            for w in range(W_WIN):
                edge_window(w)
                if w % 4 == 3:
                    node_group(list(range(w - 3, w + 1)))
            node_group([W_WIN - 1])

    nc.compile()
    return nc


def kernel(x, edge_index, edge_attr, u, batch,
           w1a, b1a, w1b, b1b, w2a, b2a, w2b, b2b, _trace=False):
    xslab, easlab, pslab8, invdn, xT = _preprocess(x, edge_index, edge_attr)

    b1a = np.ascontiguousarray(np.asarray(b1a, np.float32))
    with_b1a = bool(np.any(b1a != 0))
    key = ("nc", with_b1a)
    if key not in _cache:
        _cache[key] = _build_program(with_b1a)
    nc = _cache[key]

    wmap = dict(
        w1a=np.ascontiguousarray(np.asarray(w1a, np.float32)),
        b1a=b1a,
        w1b=np.ascontiguousarray(np.asarray(w1b, np.float32)),
        b1b=np.ascontiguousarray(np.asarray(b1b, np.float32)),
        w2a=np.ascontiguousarray(np.asarray(w2a, np.float32)),
        b2a=np.ascontiguousarray(np.asarray(b2a, np.float32)),
        w2b=np.ascontiguousarray(np.asarray(w2b, np.float32)),
        b2b=np.ascontiguousarray(np.asarray(b2b, np.float32)),
    )
    in_maps = []
    for c in range(N_CORES):
        m = dict(wmap)
        m["xslab"] = xslab[c]
        m["easlab"] = easlab[c]
        m["pslab8"] = pslab8[c]
        m["invdn"] = invdn[c]
        m["xT"] = xT[c]
        in_maps.append(m)

    res = bass_utils.run_bass_kernel_spmd(
        nc, in_maps, core_ids=list(range(N_CORES)), trace=_trace)
    kernel._last_res = res

    out = np.empty((N_CORES * NPC, D_OUT), np.float32)
    for c in range(N_CORES):
        out[c * NPC:(c + 1) * NPC] = res.results[c]["out"]
    return out[:N_NODES]


# revision 12
# speedup vs baseline: 1.0769x; 1.0290x over previous
"""GNN message-passing (NodeModel) kernel for 8 Trainium2 NeuronCores.

Strategy (edge-parallel + node-parallel, zero collectives):
  - Host: sort edges by destination; nodes split into 128-node windows
    (8 cores x 49 windows). Window edges packed into T_SUB=17 subtiles of
    128 slots. Host gathers x[row] per edge and ships feature-major slabs,
    plus per-window selection matrices P (one-hot dest with 1/deg folded
    in) so the device runs pure matmul pipelines.
  - Device per core, per window: edge MLP layer 1 in edge-major layout
    (lhsT = slab subtile), ELU via exp/relu/min identity, then the
    scatter-mean as P^T @ elu1 accumulated in PSUM. Because layer 2 of the
    edge MLP is linear, it commutes with the (linear) mean aggregation:
    w1b is applied to the 128 aggregated node vectors per window instead
    of 2176 edge vectors (17x less work). The elu(x)+1 shift used by the
    ELU identity is corrected through w1b's column sums, folded into the
    node-MLP bias on device.
  - All matmuls run in float32r (full-rate fp32 / TF32-class mode).
  - Node MLP: feature-major, ELU same trick, final layer node-major.

The kernel takes FULL inputs and returns the FULL [50000, 256] output.
"""
import numpy as np

import concourse.bass as bass
import concourse.bacc as bacc
import concourse.tile as tile
import concourse.mybir as mybir
from concourse import bass_utils
from concourse.masks import make_identity

P = 128
N_NODES = 50000
HID = 256
D_OUT = 256
N_CORES = 8

W_WIN = 49            # windows per core
T_SUB = 17            # 128-edge subtiles per window
B = T_SUB * P         # edge slots per window (2176)
EPC = W_WIN * B       # edge slots per core
NPC = W_WIN * P       # node slots per core (6272)
N_PAIR = (T_SUB + 1) // 2   # 9 psum pairs per window (last is half)
N_STT = 4             # chunks using the relu+stt variant (ACT/DVE balance)

F32 = mybir.dt.float32
FR = mybir.dt.float32r
BF = mybir.dt.bfloat16

_cache = {}


def _preprocess(x, edge_index, edge_attr):
    x = np.ascontiguousarray(np.asarray(x, dtype=np.float32))
    ea = np.ascontiguousarray(np.asarray(edge_attr, dtype=np.float32))
    ei = np.asarray(edge_index)
    row = ei[0].astype(np.int64)
    col = ei[1].astype(np.int64)
    E = row.shape[0]

    order = np.argsort(col, kind="stable")
    col_s = col[order]
    row_s = row[order]

    win = col_s >> 7
    n_win_tot = N_CORES * W_WIN
    wc = np.bincount(win, minlength=n_win_tot)
    assert wc.max() <= B, f"window overflow: {wc.max()} > {B}"
    wstart = np.zeros(n_win_tot + 1, np.int64)
    np.cumsum(wc, out=wstart[1:])
    rank = np.arange(E, dtype=np.int64) - wstart[win]

    slot = win * B + rank
    TOT = n_win_tot * B

    deg = np.bincount(col, minlength=N_NODES)
    invdeg = np.where(deg > 0, 1.0 / np.maximum(deg, 1), 0.0).astype(np.float32)

    xg = np.zeros((TOT, P), np.float32)
    xg[slot] = x[row_s]
    xslab = xg.reshape(N_CORES, EPC, P).transpose(0, 2, 1).copy()
    del xg

    eag = np.zeros((TOT, P), np.float32)
    eag[slot] = ea[order]
    easlab = eag.reshape(N_CORES, EPC, P).transpose(0, 2, 1).copy()
    del eag

    # P_t [128 edge, 128 node] one-hot (uint8), 1/deg applied at m1 copy
    psl2 = np.zeros((n_win_tot, T_SUB, P, P), np.uint8)
    e_t = (rank >> 7)
    e_p = (rank & 127)
    psl2[win, e_t, e_p, col_s - (win << 7)] = 1
    pslab8 = np.ascontiguousarray(
        psl2.transpose(0, 2, 1, 3).reshape(N_CORES, W_WIN, P, T_SUB * P))
    del psl2
    # invdn[win, n] = 1/deg of local node n
    nodepad = np.zeros(N_CORES * NPC, np.float32)
    nodepad[:N_NODES] = invdeg
    invdn = nodepad.reshape(N_CORES, W_WIN, P)

    xpad = np.zeros((N_CORES * NPC, P), np.float32)
    xpad[:N_NODES] = x
    xT = xpad.reshape(N_CORES, NPC, P).transpose(0, 2, 1).copy()

    return xslab, easlab, pslab8, invdn, xT


def _build_program(with_b1a):
    nc = bacc.Bacc("TRN2", target_bir_lowering=False, debug=False,
                   enable_asserts=False)

    xs_d = nc.dram_tensor("xslab", [P, EPC], FR, kind="ExternalInput")
    ea_d = nc.dram_tensor("easlab", [P, EPC], FR, kind="ExternalInput")
    ps_d = nc.dram_tensor("pslab8", [W_WIN, P, B], mybir.dt.uint8, kind="ExternalInput")
    ivn_d = nc.dram_tensor("invdn", [W_WIN, P], F32, kind="ExternalInput")
    xT_d = nc.dram_tensor("xT", [P, NPC], FR, kind="ExternalInput")
    w1a_d = nc.dram_tensor("w1a", [2 * P, HID], F32, kind="ExternalInput")
    b1a_d = nc.dram_tensor("b1a", [HID], F32, kind="ExternalInput")
    w1b_d = nc.dram_tensor("w1b", [HID, HID], F32, kind="ExternalInput")
    b1b_d = nc.dram_tensor("b1b", [HID], F32, kind="ExternalInput")
    w2a_d = nc.dram_tensor("w2a", [3 * P, HID], F32, kind="ExternalInput")
    b2a_d = nc.dram_tensor("b2a", [HID], F32, kind="ExternalInput")
    w2b_d = nc.dram_tensor("w2b", [HID, D_OUT], F32, kind="ExternalInput")
    b2b_d = nc.dram_tensor("b2b", [D_OUT], F32, kind="ExternalInput")
    out_d = nc.dram_tensor("out", [NPC, D_OUT], F32, kind="ExternalOutput")

    AF = mybir.ActivationFunctionType
    OP = mybir.AluOpType

    with tile.TileContext(nc) as tc:
        with (
            tc.tile_pool(name="const", bufs=1) as cp,
            tc.tile_pool(name="slab", bufs=2) as slabp,
            tc.tile_pool(name="el1", bufs=2) as el1p,
            tc.tile_pool(name="scr", bufs=4) as scrp,
            tc.tile_pool(name="m1", bufs=1) as m1p,
            tc.tile_pool(name="node", bufs=2) as nodep,
            tc.tile_pool(name="outp", bufs=3) as outp,
            tc.tile_pool(name="ps_l1", bufs=4, space="PSUM") as ps_l1,
            tc.tile_pool(name="ps_l2", bufs=2, space="PSUM") as ps_l2,
            tc.tile_pool(name="ps_agg", bufs=2, space="PSUM") as ps_agg,
        ):
            # ================= setup =================
            ident = cp.tile([P, P], F32)
            make_identity(nc, ident[:])

            ones_f = cp.tile([P, 2], F32)
            nc.vector.memset(ones_f[:], 1.0)
            ones_fr = cp.tile([P, 2], FR)
            nc.vector.tensor_copy(out=ones_fr[:], in_=ones_f[:])

            w1a = [cp.tile([P, HID], FR, name=f"w1a{k}") for k in range(2)]
            w1b = [cp.tile([P, HID], FR, name=f"w1b{k}") for k in range(2)]
            w2a = [cp.tile([P, HID], FR, name=f"w2a{k}") for k in range(3)]
            w2b = [cp.tile([P, D_OUT], FR, name=f"w2b{k}") for k in range(2)]
            for k in range(2):
                nc.gpsimd.dma_start(out=w1a[k][:], in_=w1a_d[k * P:(k + 1) * P, :])
                nc.gpsimd.dma_start(out=w1b[k][:], in_=w1b_d[k * P:(k + 1) * P, :])
                nc.gpsimd.dma_start(out=w2b[k][:], in_=w2b_d[k * P:(k + 1) * P, :])
            for k in range(3):
                nc.gpsimd.dma_start(out=w2a[k][:], in_=w2a_d[k * P:(k + 1) * P, :])

            b1b_v = [cp.tile([P, 1], F32, name=f"b1b_v{k}") for k in range(2)]
            b2a_v = [cp.tile([P, 1], F32, name=f"b2a_v{k}") for k in range(2)]
            for k in range(2):
                nc.sync.dma_start(out=b1b_v[k][:], in_=b1b_d[k * P:(k + 1) * P, None])
                nc.sync.dma_start(out=b2a_v[k][:], in_=b2a_d[k * P:(k + 1) * P, None])
            b2b_row = cp.tile([1, D_OUT], F32)
            nc.sync.dma_start(out=b2b_row[:], in_=b2b_d[None, :])

            if with_b1a:
                # b1a as a [1, 2*HID] row (doubled) for the K=1 bias matmul
                b1a_row2 = cp.tile([1, 2 * HID], FR)
                for rep in range(2):
                    nc.gpsimd.dma_start(
                        out=b1a_row2[:, rep * HID:(rep + 1) * HID],
                        in_=b1a_d[None, :])
                ones_row_f = cp.tile([1, P], F32)
                nc.vector.memset(ones_row_f[:], 1.0)
                ones_row = cp.tile([1, P], FR)
                nc.vector.tensor_copy(out=ones_row[:], in_=ones_row_f[:])

            # cvec = b1b - colsum(w1b)  (for the elu+1 shift correction)
            cvec = [cp.tile([P, 2], FR, name=f"cvec{j}") for j in range(2)]
            for j in range(2):
                cs_ps = ps_l2.tile([P, 2], F32, tag="ps_l2")
                for k in range(2):
                    nc.tensor.matmul(out=cs_ps[:], lhsT=w1b[k][:, j * P:(j + 1) * P],
                                     rhs=ones_fr[:], start=(k == 0), stop=(k == 1))
                nc.vector.tensor_tensor(out=cvec[j][:],
                                        in0=b1b_v[j][:].to_broadcast([P, 2]),
                                        in1=cs_ps[:], op=OP.subtract)

            # b2a_eff = b2a + w2a[128:384].T @ cvec
            b2a_eff = [cp.tile([P, 1], F32, name=f"b2a_eff{m}") for m in range(2)]
            b2a_eff_p1 = [cp.tile([P, 1], F32, name=f"b2a_eff_p1{m}") for m in range(2)]
            for m in range(2):
                be_ps = ps_l2.tile([P, 2], F32, tag="ps_l2")
                for k in range(2):
                    nc.tensor.matmul(out=be_ps[:], lhsT=w2a[1 + k][:, m * P:(m + 1) * P],
                                     rhs=cvec[k][:], start=(k == 0), stop=(k == 1))
                nc.vector.tensor_tensor(out=b2a_eff[m][:], in0=b2a_v[m][:],
                                        in1=be_ps[:, 0:1], op=OP.add)
                nc.vector.tensor_scalar_add(out=b2a_eff_p1[m][:], in0=b2a_eff[m][:],
                                            scalar1=1.0)

            # b2b_eff broadcast [P, 256] = b2b - colsum(w2b)
            csb_ps = ps_l2.tile([1, D_OUT], F32, tag="ps_l2")
            for k in range(2):
                nc.tensor.matmul(out=csb_ps[:], lhsT=ones_fr[:, 0:1], rhs=w2b[k][:],
                                 start=(k == 0), stop=(k == 1))
            b2b_eff_row = cp.tile([1, D_OUT], FR)
            nc.vector.tensor_tensor(out=b2b_eff_row[:], in0=b2b_row[:], in1=csb_ps[:],
                                    op=OP.subtract)
            onesc_f = cp.tile([1, P], F32, name="onesc_f")
            nc.vector.memset(onesc_f[:], 1.0)
            onesc = cp.tile([1, P], FR, name="onesc")
            nc.vector.tensor_copy(out=onesc[:], in_=onesc_f[:])
            bb_ps = ps_l1.tile([P, D_OUT], F32, tag="ps_l1")
            nc.tensor.matmul(out=bb_ps[:], lhsT=onesc[:], rhs=b2b_eff_row[:],
                             start=True, stop=True)
            b2b_bc = cp.tile([P, D_OUT], F32)
            nc.scalar.copy(out=b2b_bc[:], in_=bb_ps[:])

            m1_buf = m1p.tile([P, W_WIN * HID], F32)

            # ================= per-window edge phase =================
            def edge_window(w):
                xs = slabp.tile([P, B], FR, tag="xs", name=f"xs{w}")
                nc.sync.dma_start(out=xs[:], in_=xs_d[:, w * B:(w + 1) * B])
                eas = slabp.tile([P, B], FR, tag="eas", name=f"eas{w}")
                nc.sync.dma_start(out=eas[:], in_=ea_d[:, w * B:(w + 1) * B])
                ivn = slabp.tile([P, 1], F32, tag="ivn", name=f"ivn{w}")
                nc.sync.dma_start(out=ivn[:], in_=ivn_d[w, :, None])
                psl = slabp.tile([P, B], BF, tag="psl", name=f"psl{w}")
                nc.gpsimd.dma_start(out=psl[:], in_=ps_d[w, :, :])

                el1 = el1p.tile([P, T_SUB * HID], BF, tag="el1", name=f"el1_{w}")
                for pr in range(N_PAIR):
                    t0 = 2 * pr
                    nsub = 1 if t0 == T_SUB - 1 else 2
                    pw = nsub * HID
                    h1 = ps_l1.tile([P, 512], F32, tag="ps_l1", name=f"h1_{w}_{pr}")
                    if with_b1a:
                        nc.tensor.matmul(out=h1[:, :pw], lhsT=ones_row[:],
                                         rhs=b1a_row2[:, :pw], start=True, stop=False)
                    for ti in range(nsub):
                        t = t0 + ti
                        for k in range(2):
                            slab = xs if k == 0 else eas
                            nc.tensor.matmul(
                                out=h1[:, ti * HID:(ti + 1) * HID],
                                lhsT=slab[:, t * P:(t + 1) * P],
                                rhs=(w1a[k][:]),
                                start=(k == 0 and not with_b1a),
                                stop=(k == 1))
                    seg = el1[:, t0 * HID: t0 * HID + pw]
                    if pr < N_PAIR - N_STT:
                        # variant A: exp (ACT) -> r1 (DVE ts) -> min (DVE tt)
                        nc.scalar.activation(out=seg, in_=h1[:, :pw], func=AF.Exp)
                        r1 = scrp.tile([P, 512], F32, tag="scr", name=f"r1_{w}_{pr}")
                        nc.vector.tensor_scalar(
                            out=r1[:, :pw], in0=h1[:, :pw],
                            scalar1=1.0, scalar2=1.0, op0=OP.add, op1=OP.max)
                        nc.vector.tensor_tensor(out=seg, in0=seg, in1=r1[:, :pw],
                                                op=OP.min)
                    else:
                        # variant B: exp (ACT) + relu (ACT) -> stt (DVE)
                        ex = scrp.tile([P, 512], F32, tag="scr", name=f"ex_{w}_{pr}")
                        nc.scalar.activation(out=ex[:, :pw], in_=h1[:, :pw], func=AF.Exp)
                        rl = scrp.tile([P, 512], F32, tag="scr", name=f"rl_{w}_{pr}")
                        nc.scalar.activation(out=rl[:, :pw], in_=h1[:, :pw], func=AF.Relu)
                        nc.vector.scalar_tensor_tensor(
                            out=seg, in0=rl[:, :pw], scalar=1.0, in1=ex[:, :pw],
                            op0=OP.add, op1=OP.min)

                m1 = ps_agg.tile([P, HID], F32, tag="ps_agg", name=f"m1_{w}")
                for t in range(T_SUB):
                    nc.tensor.matmul(out=m1[:],
                                     lhsT=psl[:, t * P:(t + 1) * P],
                                     rhs=el1[:, t * HID:(t + 1) * HID],
                                     start=(t == 0), stop=(t == T_SUB - 1))
                nc.scalar.activation(out=m1_buf[:, w * HID:(w + 1) * HID], in_=m1[:],
                                     func=AF.Identity, scale=ivn[:, :1])

            # ================= per-group node phase =================
            def node_group(ws):
                ncols = P * len(ws)
                base = ws[0] * P
                gi = ws[0] // 4
                xz = nodep.tile([P, 512], FR, tag="xz", name=f"xz{gi}")
                nc.sync.dma_start(out=xz[:, :ncols], in_=xT_d[:, base:base + ncols])
                # transpose m1 -> feature-major m1f
                m1f = [nodep.tile([P, 512], FR, tag=f"m1f{fh}", name=f"m1f{fh}_{gi}")
                       for fh in range(2)]
                for wi, w in enumerate(ws):
                    for fh in range(2):
                        trp = ps_l2.tile([P, P], F32, tag="ps_l2",
                                         name=f"trp{gi}_{wi}_{fh}")
                        nc.tensor.transpose(
                            out=trp[:],
                            in_=m1_buf[:, w * HID + fh * P: w * HID + (fh + 1) * P],
                            identity=ident[:])
                        nc.scalar.copy(out=m1f[fh][:, wi * P:(wi + 1) * P], in_=trp[:])
                # mean = w1b^T @ m1f  (edge-MLP layer 2 on aggregated nodes)
                zf = [nodep.tile([P, 512], FR, tag=f"zf{m}", name=f"zf{m}_{gi}")
                      for m in range(2)]
                for m in range(2):
                    mps = ps_l1.tile([P, 512], F32, tag="ps_l1", name=f"mps{gi}_{m}")
                    for k in range(2):
                        nc.tensor.matmul(out=mps[:, :ncols],
                                         lhsT=w1b[k][:, m * P:(m + 1) * P],
                                         rhs=m1f[k][:, :ncols],
                                         start=(k == 0), stop=(k == 1))
                    nc.scalar.copy(out=zf[m][:, :ncols], in_=mps[:, :ncols])
                # node MLP layer 1
                el3 = [nodep.tile([P, 512], FR, tag=f"el3{m}", name=f"el3{m}_{gi}")
                       for m in range(2)]
                for m in range(2):
                    h3 = ps_l1.tile([P, 512], F32, tag="ps_l1", name=f"h3_{gi}_{m}")
                    for k in range(3):
                        rhs = (xz if k == 0 else zf[k - 1])[:, :ncols]
                        nc.tensor.matmul(out=h3[:, :ncols],
                                         lhsT=w2a[k][:, m * P:(m + 1) * P],
                                         rhs=rhs, start=(k == 0), stop=(k == 2))
                    nc.scalar.activation(out=el3[m][:, :ncols], in_=h3[:, :ncols],
                                         func=AF.Exp, bias=b2a_eff[m][:, :1], scale=1.0)
                    r3 = scrp.tile([P, 512], F32, tag="scr", name=f"r3_{gi}_{m}")
                    nc.vector.tensor_scalar(
                        out=r3[:, :ncols], in0=h3[:, :ncols],
                        scalar1=b2a_eff_p1[m][:, :1], scalar2=1.0,
                        op0=OP.add, op1=OP.max)
                    nc.vector.tensor_tensor(out=el3[m][:, :ncols],
                                            in0=el3[m][:, :ncols], in1=r3[:, :ncols],
                                            op=OP.min)
                # node MLP layer 2 + bias, store
                for si, w in enumerate(ws):
                    o_ps = ps_l2.tile([P, D_OUT], F32, tag="ps_l2",
                                      name=f"ops{gi}_{si}")
                    for k in range(2):
                        nc.tensor.matmul(out=o_ps[:],
                                         lhsT=el3[k][:, si * P:(si + 1) * P],
                                         rhs=w2b[k][:], start=(k == 0), stop=(k == 1))
                    ob = outp.tile([P, D_OUT], F32, tag="ob", name=f"ob{gi}_{si}")
                    nc.vector.tensor_tensor(out=ob[:], in0=o_ps[:], in1=b2b_bc[:],
                                            op=OP.add)
                    nc.sync.dma_start(out=out_d[w * P:(w + 1) * P, :], in_=ob[:])

            for w in range(W_WIN):
                edge_window(w)
                if w % 4 == 3:
                    node_group(list(range(w - 3, w + 1)))
            node_group([W_WIN - 1])

    nc.compile()
    return nc


def kernel(x, edge_index, edge_attr, u, batch,
           w1a, b1a, w1b, b1b, w2a, b2a, w2b, b2b, _trace=False):
    xslab, easlab, pslab8, invdn, xT = _preprocess(x, edge_index, edge_attr)

    b1a = np.ascontiguousarray(np.asarray(b1a, np.float32))
    with_b1a = bool(np.any(b1a != 0))
    key = ("nc", with_b1a)
    if key not in _cache:
        _cache[key] = _build_program(with_b1a)
    nc = _cache[key]

    wmap = dict(
        w1a=np.ascontiguousarray(np.asarray(w1a, np.float32)),
        b1a=b1a,
        w1b=np.ascontiguousarray(np.asarray(w1b, np.float32)),
        b1b=np.ascontiguousarray(np.asarray(b1b, np.float32)),
        w2a=np.ascontiguousarray(np.asarray(w2a, np.float32)),
        b2a=np.ascontiguousarray(np.asarray(b2a, np.float32)),
        w2b=np.ascontiguousarray(np.asarray(w2b, np.float32)),
        b2b=np.ascontiguousarray(np.asarray(b2b, np.float32)),
    )
    in_maps = []
    for c in range(N_CORES):
        m = dict(wmap)
        m["xslab"] = xslab[c]
        m["easlab"] = easlab[c]
        m["pslab8"] = pslab8[c]
        m["invdn"] = invdn[c]
        m["xT"] = xT[c]
        in_maps.append(m)

    res = bass_utils.run_bass_kernel_spmd(
        nc, in_maps, core_ids=list(range(N_CORES)), trace=_trace)
    kernel._last_res = res

    out = np.empty((N_CORES * NPC, D_OUT), np.float32)
    for c in range(N_CORES):
        out[c * NPC:(c + 1) * NPC] = res.results[c]["out"]
    return out[:N_NODES]
